# revision 24
# baseline (speedup 1.0000x reference)
# MoE layer (all-experts dense MLP + weighted combine) on 8 TRN2 NeuronCores.
#
# Reference, for every token b (B=65536 total):
#   h_e   = relu(x @ W1[e] + b1[e])          e = 0..7
#   y_e   = h_e @ W2[e] + b2[e]
#   out_b = sum_e weights[b, e] * y_e
#
# Strategy (data-parallel over B, expert params replicated):
#   - Shard B across the 8 cores (8192 tokens each).
#   - Hidden dim stays on partitions ("hdim-major"):
#       L1:  z_e^T[h, b]  = W1_e^T @ x^T          (W1 chunks stationary)
#       h_e^T             = relu(z_e^T + b1_e)    (ACT, all 16 chunks/tile)
#       hs_e^T            = h_e^T * w_bcast_e     (DVE, all 16 chunks/tile)
#       out^T[o, b]       = sum_{e,k} W2_chunk^T @ hs_chunk + b2^T @ w^T
#     accumulated in two per-half PSUM banks - the expert combine is free
#     and consecutive tiles' accumulations overlap.
#   - Engine balance per tile (PE floor ~14.3us): ACT = 16 relus + 1
#     drain ~12.4us, DVE = 16 mults + 1 drain ~11.6us, GPSIMD/SYNC =
#     DMA triggers only.  Everything sits under the PE.
#   - w broadcast to 128 partitions via per-expert DMAs (step-0 partition
#     AP on the host-transposed weights row), split across the gpsimd
#     queue (e0-3) and sync queue (e4-7) so neither queue carries more
#     than ~1.1MB/tile, and PREFETCHED one tile ahead: region t triggers
#     tile t+1's wbc/xt/wt before any store, so loads never sit behind
#     the out-DMA trigger that blocks until tile t's drain.
#   - Head: consts interleaved so each dependency lands just-in-time:
#     Q0(gpsimd) = w1 pieces / w2 pieces / wbc-t0-e0..3 interleaved,
#     Q1(sync) = xt0 halves, wt0, wbc-t0-e4..7, Q10(scalar) = b1, b2.
#     Dummy matmuls on a memset tile (emitted first on DVE) spin the
#     HAM clock gate up while the head is DMA-bound.
#   - Output staged in bf16 (halves the store traffic); host converts
#     back to f32 and un-transposes.  All device work is the
#     unavoidable compute.
import numpy as np
import ml_dtypes

import concourse.bass as bass
import concourse.mybir as mybir
import concourse.tile as tile
import concourse.bass_utils as _bu
from concourse.bass_utils import run_bass_kernel_spmd


E, D_IN, D_HID, D_OUT, B = 8, 128, 256, 128, 65536
N_CORES = 8
B_SHARD = B // N_CORES  # 8192
NB = 1024               # tokens per tile
NCHUNK = D_HID // 128   # 2 hidden-dim chunks per expert

BF16 = mybir.dt.bfloat16
F32 = mybir.dt.float32
RELU = mybir.ActivationFunctionType.Relu

N_WARM_MM = 30  # dummy matmuls bridging the DMA-bound head (HAM spin-up
                # needs ~3.4us of sustained PE activity, and tile-0's xt
                # lands ~11us in; running dry re-throttles the clock gate)
# GPSIMD tensor ops measured ~1-2.5us effective for [128,1024] bf16 TT:
# far too slow to offload mults.  Keep all mults on DVE.
GP_MULT_EXPERTS = ()
# PSUM z-drain relus: ACT does 12 (~1.11us each), DVE does 4 (~1.27us)
# plus the 16 4x-rate mults (~0.33us) and both o-drains.  DVE relus sit
# on the m1 chunk of alternating experts so ACT's 2-relu experts
# interleave with 1-relu experts and the drift self-corrects.
DVE_RELU_CHUNKS = (1, 5, 9, 13)

_nc_cache = {}


def dedup_ldw(nc):
    """Drop redundant PE weight loads.

    Tile emits an InstLdweights before every InstMatmult; consecutive
    matmuls over the two 512-token halves of a tile reuse the same
    stationary weights, so the second load is a hardware no-op (weights
    persist in the PE array until the next load). Deleting it saves PE
    queue time; its semaphore waits/updates are carried onto the next PE
    instruction (legalize_waits splits any overflow afterwards).
    """
    for f in nc.m.functions:
        for b in f.blocks:
            il = b.instructions
            out = []
            last_key = None
            carry_w, carry_u = [], []
            for inst in il:
                if inst.engine != mybir.EngineType.PE:
                    out.append(inst)
                    continue
                if isinstance(inst, mybir.InstLdweights):
                    key = str(inst.ins[0])
                    if key == last_key:
                        si = inst.sync_info
                        if si is not None:
                            carry_w.extend(list(si.on_wait))
                            carry_u.extend(list(si.on_update))
                        continue
                    last_key = key
                elif not isinstance(
                    inst, (mybir.InstMatmult, mybir.InstEventSemaphore)
                ):
                    last_key = None
                if carry_w or carry_u:
                    si = inst.sync_info
                    w = (list(si.on_wait) if si else []) + carry_w
                    u = (list(si.on_update) if si else []) + carry_u
                    inst.sync_info = mybir.SyncInfo(on_wait=w, on_update=u)
                    carry_w, carry_u = [], []
                out.append(inst)
            il[:] = out
    return nc


def legalize_waits(nc):
    """Split multi-wait instructions into standalone EventSemaphore waits.

    The walrus build in this container enforces the hardware sync-slot
    budget strictly: a normal instruction holds at most 1 sem wait (+1
    update); an EventSemaphore instruction holds 2. Tile's scheduler
    attaches up to 3 waits per instruction (and ~11 on the kernel-tail
    drain), which codegen rejects with "Too many sync wait commands".
    Hoisting the excess waits into standalone EventSemaphore instructions
    immediately before the op (same engine queue, so they gate execution
    identically) makes the program legal without changing semantics.
    """
    for f in nc.m.functions:
        for b in f.blocks:
            il = b.instructions
            out = []
            changed = False
            for inst in il:
                si = inst.sync_info
                if si is not None:
                    waits = list(si.on_wait)
                    upds = list(si.on_update)
                    assert len(upds) <= 1, f"{inst.name}: {len(upds)} updates"
                    cap = 2 if isinstance(inst, mybir.InstEventSemaphore) else 1
                    if len(waits) > cap:
                        extra, keep = waits[:-cap], waits[-cap:]
                        k = 0
                        while extra:
                            chunk, extra = extra[:2], extra[2:]
                            ev = mybir.InstEventSemaphore(
                                name=f"{inst.name}-lw{k}", ins=[], outs=[]
                            )
                            ev.engine = inst.engine
                            ev.sync_info = mybir.SyncInfo(
                                on_wait=chunk, on_update=[]
                            )
                            out.append(ev)
                            k += 1
                        inst.sync_info = mybir.SyncInfo(
                            on_wait=keep, on_update=upds
                        )
                        changed = True
                out.append(inst)
            if changed:
                il[:] = out
    return nc


def build_nc(b_shard=B_SHARD, nb=NB, legalize=True):
    assert b_shard % nb == 0
    n_tiles = b_shard // nb
    nc = bass.Bass(trn_type="TRN2")

    xt = nc.dram_tensor("xt", [D_IN, b_shard], BF16, kind="ExternalInput").ap()
    wt = nc.dram_tensor("wt", [E, b_shard], BF16, kind="ExternalInput").ap()
    # W1 laid out [i, (e, m), h']: chunk (e, m) is lhsT for z_e rows m*128..
    w1l = nc.dram_tensor("w1l", [D_IN, E * NCHUNK, 128], BF16, kind="ExternalInput").ap()
    # b1 laid out [p, (e, m)] = b1[e, m*128 + p]
    b1l = nc.dram_tensor("b1l", [128, E * NCHUNK], F32, kind="ExternalInput").ap()
    # W2 laid out [h', (e, k), o]: chunk (e, k) is lhsT contracting h rows k*128..
    w2l = nc.dram_tensor("w2l", [128, E * NCHUNK, D_OUT], BF16, kind="ExternalInput").ap()
    b2p = nc.dram_tensor("b2p", [E, D_OUT], BF16, kind="ExternalInput").ap()
    outT = nc.dram_tensor("outT", [D_OUT, b_shard], BF16, kind="ExternalOutput").ap()

    nsub = nb // 512  # matmul moving-operand splits per tile
    with tile.TileContext(nc) as tc:
        with (
            tc.tile_pool(name="consts", bufs=1) as consts,
            tc.tile_pool(name="xt_p", bufs=3) as xt_p,
            tc.tile_pool(name="wt_p", bufs=3) as wt_p,
            tc.tile_pool(name="wbc_p", bufs=3) as wbc_p,
            tc.tile_pool(name="h_p", bufs=8) as h_p,
            tc.tile_pool(name="hs_p", bufs=8) as hs_p,
            tc.tile_pool(name="ot_p", bufs=3) as ot_p,
            # PSUM budget: 8 banks of 2KB. z gets 3 full tiles (6 banks);
            # the output accumulator is 2 independent half-tiles (1 bank
            # each) so consecutive tiles' accumulations can overlap.
            tc.tile_pool(name="z_ps", bufs=3, space="PSUM") as z_ps,
            tc.tile_pool(name="o_ps", bufs=2, space="PSUM") as o_ps,
        ):
            # --- head: consts + tile-0 inputs, interleaved just-in-time ---
            # scalar queue: ALL triggers first (the b1 launder copy would
            # otherwise head-of-line block the w2 trigger until b1 lands);
            # w2 is first needed by L2-e0 at ~11us, 512KB lands ~9us.
            b1_dma = consts.tile([128, E * NCHUNK], F32, tag="b1_dma")
            nc.scalar.dma_start(b1_dma, b1l)
            b2_sb = consts.tile([E, D_OUT], BF16)
            nc.scalar.dma_start(b2_sb, b2p)
            w2_sb = consts.tile([128, E * NCHUNK, D_OUT], BF16)
            nc.scalar.dma_start(w2_sb, w2l)
            b1_sb = consts.tile([128, E * NCHUNK], F32, tag="b1_act")
            nc.scalar.copy(b1_sb, b1_dma)
            w1_sb = consts.tile([D_IN, E * NCHUNK, 128], BF16)

            def wbc_dma(eng, wbc_t, t, e):
                # broadcast one weight row to 128 partitions via a step-0
                # partition AP on the host-transposed weights in DRAM
                # (SBUF sources cannot have a zero partition step)
                eng.dma_start(
                    wbc_t[:, e, :],
                    wt[e : e + 1, t * nb : (t + 1) * nb].partition_broadcast(128),
                )

            def wbc_dma_all(eng, wbc_t, t):
                # whole-tile broadcast in ONE transfer: 128 descriptors of
                # 16KB (2D-strided src) instead of 8x128 of 2KB - the
                # per-expert form bottoms out at ~75GB/s on descriptor
                # overhead and saturates the queues.
                eng.dma_start(
                    wbc_t[:, :, :],
                    bass.AP(
                        tensor=wt.tensor,
                        offset=t * nb,
                        ap=[[0, 128], [b_shard, E], [1, nb]],
                    ),
                )

            # tile-0 inputs: the head is HBM-aggregate-bound (~3.5MB with
            # the 2.1MB broadcast), spread just-in-time over all queues:
            # sync = xt halves (gate the first matmul), wt, wbc e2-5;
            # scalar (slow queue, tiny+w2 only) also takes wbc e6-7 late;
            # gpsimd = w1 pieces interleaved with wbc e0-1.
            xt_sb = xt_p.tile([D_IN, nb], BF16)
            for j in range(nsub):
                nc.sync.dma_start(
                    xt_sb[:, j * 512 : (j + 1) * 512], xt[:, j * 512 : (j + 1) * 512]
                )
            wt_sb = wt_p.tile([E, nb], BF16)
            nc.sync.dma_start(wt_sb, wt[:, 0:nb])
            wbc = wbc_p.tile([128, E, nb], BF16)
            # broadcast need-times are ~12.5+1.5k us for expert k; the
            # slow scalar queue can only absorb the last two.
            for e in (0, 1, 2):
                wbc_dma(nc.sync, wbc, 0, e)
            wbc_dma(nc.scalar, wbc, 0, 6)
            wbc_dma(nc.scalar, wbc, 0, 7)
            nc.gpsimd.dma_start(w1_sb[:, 0:2, :], w1l[:, 0:2, :])
            nc.gpsimd.dma_start(w1_sb[:, 2:4, :], w1l[:, 2:4, :])
            nc.gpsimd.dma_start(w1_sb[:, 4:10, :], w1l[:, 4:10, :])
            nc.gpsimd.dma_start(w1_sb[:, 10:16, :], w1l[:, 10:16, :])
            for e in (3, 4, 5):
                wbc_dma(nc.gpsimd, wbc, 0, e)

            # DVE-side b1 copy for the DVE relu chunks; emitted after the
            # warm memset so the vector FIFO isn't head-of-line blocked
            # on the b1 DMA before the warm-up can run.

            # Pre-warm the PE while the head is DMA-bound: the HAM clock
            # gate starts at 1.2 GHz and needs ~3.4us of sustained matmul
            # activity to release to 2.4 GHz. Burn that window on dummy
            # matmuls over a memset tile so tile 0's real matmuls run warm.
            if N_WARM_MM:
                warm = consts.tile([128, 128], BF16, tag="warm")
                nc.vector.memset(warm, 0)
                zw = z_ps.tile([128, nb], F32, tag="z")
                for _ in range(N_WARM_MM):
                    nc.tensor.matmul(
                        zw[:, :128], lhsT=warm, rhs=warm,
                        start=True, stop=True,
                    )
            b1v_sb = consts.tile([128, E * NCHUNK], F32, tag="b1_dve")
            nc.vector.tensor_copy(b1v_sb, b1_dma)

            for t in range(n_tiles):
                b0 = t * nb
                # ---- prefetch tile t+1 inputs (loads BEFORE this tile's
                # stores on each queue, so they never sit behind the
                # out-DMA trigger that blocks until tile t drains) ----
                if t + 1 < n_tiles:
                    b1n_ = (t + 1) * nb
                    xt_nx = xt_p.tile([D_IN, nb], BF16)
                    nc.sync.dma_start(xt_nx, xt[:, b1n_ : b1n_ + nb])
                    wt_nx = wt_p.tile([E, nb], BF16)
                    nc.sync.dma_start(wt_nx, wt[:, b1n_ : b1n_ + nb])
                    wbc_nx = wbc_p.tile([128, E, nb], BF16)
                    for e in range(E):
                        eng = nc.gpsimd if e < 4 else nc.sync
                        wbc_dma(eng, wbc_nx, t + 1, e)
                else:
                    xt_nx = wt_nx = wbc_nx = None

                # out^T accumulates per 512-token half in its own PSUM bank;
                # the b2' seed matmuls are emitted after expert 0's z work so
                # the PE has useful work while the previous tile's bank drains
                pos = []
                for j in range(nsub):
                    po = o_ps.tile([D_OUT, 512], F32, tag="po")
                    pos.append(po)

                def flush(pe, phs, ph):
                    # multiply + L2 for an expert whose relus were emitted
                    # one expert ago: the h/wbc deps get a full expert
                    # period of slack before the mult must run.
                    eng = nc.gpsimd if pe in GP_MULT_EXPERTS else nc.vector
                    for m in range(NCHUNK):
                        eng.tensor_mul(
                            phs[:, m, :], ph[:, m, :], wbc[:, pe, :]
                        )
                    for k in range(NCHUNK):
                        c = NCHUNK * pe + k
                        for j in range(nsub):
                            sl = slice(j * 512, (j + 1) * 512)
                            nc.tensor.matmul(
                                pos[j], lhsT=w2_sb[:, c, :], rhs=phs[:, k, sl],
                                start=False,
                                stop=(pe == E - 1 and k == NCHUNK - 1),
                            )

                pend = None
                t0_pends = []
                for e in range(E):
                    hs = hs_p.tile([128, NCHUNK, nb], BF16)
                    h = h_p.tile([128, NCHUNK, nb], BF16)
                    zs = []
                    for m in range(NCHUNK):
                        c = NCHUNK * e + m
                        z = z_ps.tile([128, nb], F32, tag="z")
                        zs.append(z)
                        for j in range(nsub):
                            sl = slice(j * 512, (j + 1) * 512)
                            nc.tensor.matmul(
                                z[:, sl], lhsT=w1_sb[:, c, :], rhs=xt_sb[:, sl],
                                start=True, stop=True,
                            )
                    if e == 1:
                        # out^T := b2'^T @ w^T   (K = 8), opens the group.
                        # Emitted after z-e1 (not e0): the o-PSUM banks
                        # rotate 2-deep, so the seed must not reach the PE
                        # before the previous tile's drain copies finish.
                        for j in range(nsub):
                            sl = slice(j * 512, (j + 1) * 512)
                            nc.tensor.matmul(
                                pos[j], lhsT=b2_sb, rhs=wt_sb[:, sl],
                                start=True, stop=False,
                            )
                    # flush the PREVIOUS expert before emitting this
                    # expert's relus: the DVE FIFO then runs [mults e-1,
                    # relu e] — the mults' deps are already satisfied, so
                    # the L2 matmuls (which wait on them) start sooner.
                    # Tile 0 is special: its wbc lands late (HBM-bound
                    # head), and an early flush head-of-line blocks the
                    # remaining ready z matmuls in the PE FIFO for ~5us,
                    # re-throttling the HAM clock gate.  Defer ALL its
                    # flushes past the z/relu stream (h/hs pools hold
                    # exactly 8 live experts).
                    if pend is not None:
                        if t == 0:
                            t0_pends.append(pend)
                        else:
                            flush(*pend)
                    for m in range(NCHUNK):
                        c = NCHUNK * e + m
                        if c in DVE_RELU_CHUNKS:
                            # DVE relu: (z + b1) max 0, cast to bf16
                            nc.vector.tensor_scalar(
                                h[:, m, :], zs[m],
                                b1v_sb[:, c : c + 1], 0.0,
                                mybir.AluOpType.add, mybir.AluOpType.max,
                            )
                        else:
                            nc.scalar.activation(
                                h[:, m, :], zs[m], RELU,
                                bias=b1_sb[:, c : c + 1], scale=1.0,
                            )
                    pend = (e, hs, h)
                if t == 0:
                    for p in t0_pends:
                        flush(*p)
                flush(*pend)

                # drain: both halves via ACT (DVE carries 4 relus + all
                # 16 mults and is the pacing engine), staged bf16.
                ot = ot_p.tile([D_OUT, nb], BF16)
                nc.scalar.copy(ot[:, 0:512], pos[0])
                nc.scalar.copy(ot[:, 512:1024], pos[1])
                for j in range(nsub):
                    sl = slice(j * 512, (j + 1) * 512)
                    nc.sync.dma_start(outT[:, b0 + j * 512 : b0 + (j + 1) * 512], ot[:, sl])

                xt_sb, wt_sb, wbc = xt_nx, wt_nx, wbc_nx
    dedup_ldw(nc)
    return legalize_waits(nc) if legalize else nc


def prep_consts(W1, b1, W2, b2):
    bf = ml_dtypes.bfloat16
    # w1l[i, (e, m), h'] = W1[e, i, m*128 + h']
    w1l = np.ascontiguousarray(
        W1.transpose(1, 0, 2).reshape(D_IN, E, NCHUNK, 128).reshape(D_IN, E * NCHUNK, 128)
    ).astype(bf)
    # b1l[p, (e, m)] = b1[e, m*128 + p]
    b1l = np.ascontiguousarray(
        b1.reshape(E, NCHUNK, 128).transpose(2, 0, 1).reshape(128, E * NCHUNK)
    ).astype(np.float32)
    # w2l[h', (e, k), o] = W2[e, k*128 + h', o]
    w2l = np.ascontiguousarray(
        W2.reshape(E, NCHUNK, 128, D_OUT).transpose(2, 0, 1, 3).reshape(128, E * NCHUNK, D_OUT)
    ).astype(bf)
    return {
        "w1l": w1l,
        "b1l": b1l,
        "w2l": w2l,
        "b2p": b2.astype(np.float32).astype(bf),
    }


def prep_core(x_c, w_c, consts, b_shard):
    bf = ml_dtypes.bfloat16
    xt = np.ascontiguousarray(x_c.T).astype(bf)
    wt = np.ascontiguousarray(w_c.T).astype(bf)
    return {"xt": xt, "wt": wt, **consts}


def _ntff_hook():
    """NTFF profiling hook via the axon PJRT .so (the antenv.axon_hooks
    glue module is absent in this image, so wire it up directly)."""
    from trn_agent_boot.trn_boot import _ntff_profile_via_ctypes

    return _ntff_profile_via_ctypes("/opt/axon/libaxon_pjrt.so")


def run_traced(nc, in_maps, n_cores, out_dir):
    import concourse.bass2jax as bass2jax

    hook = _ntff_hook()
    with hook(out_dir, list(range(n_cores))):
        results = bass2jax.run_bass_via_pjrt(nc, in_maps, n_cores=n_cores)
    return results


def run(inputs, trace=False, b_shard=B_SHARD, nb=NB):
    x = np.asarray(inputs["x"], dtype=np.float32)
    w = np.asarray(inputs["weights"], dtype=np.float32)
    consts = prep_consts(
        np.asarray(inputs["W1"], dtype=np.float32),
        np.asarray(inputs["b1"], dtype=np.float32),
        np.asarray(inputs["W2"], dtype=np.float32),
        np.asarray(inputs["b2"], dtype=np.float32),
    )
    n_cores = x.shape[0] // b_shard
    key = (b_shard, nb)
    if key not in _nc_cache:
        _nc_cache[key] = build_nc(b_shard, nb)
    nc = _nc_cache[key]
    in_maps = [
        prep_core(
            x[c * b_shard : (c + 1) * b_shard],
            w[c * b_shard : (c + 1) * b_shard],
            consts,
            b_shard,
        )
        for c in range(n_cores)
    ]
    if trace:
        import tempfile

        out_dir = tempfile.mkdtemp(prefix="moe_ntff_")
        results = run_traced(nc, in_maps, n_cores, out_dir)

        class _Res:
            pass

        res = _Res()
        res.results = results
        res.exec_time_ns = None
        res.trace_dir = out_dir
    else:
        res = run_bass_kernel_spmd(
            nc, in_maps, core_ids=list(range(n_cores)), trace=False
        )
        res.trace_dir = None
    out = np.concatenate(
        [np.ascontiguousarray(r["outT"].T).astype(np.float32) for r in res.results],
        axis=0,
    )
    return out, res


def kernel(**inputs) -> np.ndarray:
    out, _ = run(inputs)
    return out
